# revision 32
# baseline (speedup 1.0000x reference)
"""Multi-head graph attention (GAT) Trainium2 kernel, 8-core SPMD.

Problem: h[4096,256], adj[4096,4096] bool, w[4,256,64], a_src/a_dst[4,64,1],
bias[64] -> out[4096,4,64]:
    h_prime = h @ w[k]                       per head
    s[i,j]  = src[i] + dst[j]                (rank-1!)
    scores  = leaky_relu(s, 0.2), masked by adj, softmax over j
    out     = attn @ h_prime + bias

Sharding: 8 cores = 2 head-groups x 4 row-blocks. Core c computes heads
[2*(c%2), 2*(c%2)+1] for output rows [1024*(c//2), 1024*(c//2)+1024).

Key algebra (all on-device, per head):
    exp(leaky(s)) = max(e^s, e^{0.2 s}) = e^{0.2 s} * max(e^{0.8 s}, 1)
    e^{0.2 s} = e^{0.2 src_i} * e^{0.2 dst_j};  e^{0.8 s} factors likewise.
The per-column factor e^{0.2 src_i} cancels in the softmax normalization,
so the unnormalized weights reduce to
    P'[j,i] = e^{0.2 dst_j - C} * adj[i,j] * max(e^{0.8 src_i} * e^{0.8 dst_j}, 1)
which is two elementwise ops per tile: a dual-op tensor_scalar
(mult + max-with-1, DVE 4x perf mode) on a replicated e^{0.8 src} tile,
and one mask tensor_tensor (DVE 2x mode).  A tunable subset of the mask
multiplies runs on the otherwise-idle GpSimd engine (~2.2us/tile there vs
~0.7us on DVE) to unload the DVE critical path.  The row factor
e^{0.2 dst - C} rides the stationary bmm operand G = [h' * f | f]
(f = e^{0.2 dst - C} written once into G's 65th column and reused as the
ACT scale operand, so it cancels exactly in the epilogue division).
"""

import sys

sys.path.insert(0, "/opt/trn_rl_repo")

import numpy as np
import ml_dtypes

N = 4096          # nodes
F = 256           # f_in
O = 64            # f_out
NHEAD = 4
NCORES = 8
NH = 2            # heads per core
NI = 1024         # output rows per core
NCJ = N // 128    # 32 j-chunks
NSEG = NI // 512  # 2 segments of 512 in the i (free) dim
NSUB = NI // 128  # 8 i-subtiles of 128
NF = N - NI       # 3072 j-columns outside this core's i-block
NCI = NI // 128   # 8 j-chunks whose h columns come from the hTi tile
CSH = 2.0         # shift inside e^{0.2 dst - CSH} (dst is within +-9)
GW = 68           # per-(chunk,head) G stride: 64 h_prime + factor col + pad

# (chunk, head) tiles whose mask multiply runs on GpSimd instead of DVE:
# tile index (2c + h) mod POOL_MOD < POOL_CNT.  NOTE: measured on HW, GpSimd
# elementwise ops trigger heavy chip-wide duty-cycle throttling (HAM 4/8
# windows) that slows every other engine -- keep POOL_CNT = 0.
POOL_MOD = 8
POOL_CNT = 0

_CACHE = {}


def _build():
    import concourse.bass as bass
    import concourse.bacc as bacc
    import concourse.mybir as mybir
    import concourse.tile as tile
    from concourse.bass import ts

    from concourse.masks import make_identity

    f32 = mybir.dt.float32
    bf16 = mybir.dt.bfloat16
    Alu = mybir.AluOpType
    Act = mybir.ActivationFunctionType

    nc = bacc.Bacc()
    fp16 = mybir.dt.float16
    hTf_d = nc.declare_dram_parameter("hTf", [F, NF], bf16, isOutput=False)
    hTi_d = nc.declare_dram_parameter("hTi", [F, NI], bf16, isOutput=False)
    adjT_d = nc.declare_dram_parameter("adjT", [8 * 128, 4 * NI], bf16, isOutput=False)
    wr_d = nc.declare_dram_parameter("wr", [F, NH * O], bf16, isOutput=False)
    wta_d = nc.declare_dram_parameter("wta", [O, NH * F + 2 * NH], bf16, isOutput=False)
    out_d = nc.declare_dram_parameter("out", [NH, 128, NSUB * O], fp16, isOutput=True)

    with tile.TileContext(nc) as tc:
        with (
            tc.tile_pool(name="sb", bufs=1) as sb,
            tc.tile_pool(name="sbr", bufs=2) as sbr,
            tc.tile_pool(name="sbo", bufs=3) as sbo,
            tc.tile_pool(name="pw", bufs=4, space="PSUM") as pw,
            tc.tile_pool(name="pacc", bufs=1, space="PSUM") as pacc,
        ):
            # ---- static SBUF tensors ----
            hT_sb = sb.tile([128, 2, NF], bf16, name="hT_sb")
            hTi_sb = sb.tile([128, 2, NI], bf16, name="hTi_sb")
            adjT_sb = sb.tile([128, NCJ, NI], bf16, name="adjT_sb")
            wta_sb = sb.tile([O, NH * F + 2 * NH], bf16, name="wta_sb")
            wTr_sb = wta_sb[:, 0 : NH * F].rearrange("o (h f) -> o h f", h=NH)
            avec_sb = wta_sb[:, NH * F : NH * F + 2 * NH]
            wall_sb = sb.tile([128, 2, NH * O + NH], bf16, name="wall_sb")
            vsrc_sb = sb.tile([128, 2, NH], bf16, name="vsrc_sb")
            ones_row = sb.tile([1, 128], bf16, name="ones_row")
            esrc3_rep = sb.tile([128, NH, NI], bf16, name="esrc3_rep")
            esrc3_row = sb.tile([1, NH, NI], bf16, name="esrc3_row")
            g2_sb = sb.tile([128, NCJ, NH, GW], bf16, name="g2_sb")
            edst3_sb = sb.tile([128, NCJ, NH], f32, name="edst3_sb")
            edst2_sb = sb.tile([128, NCJ, NH], f32, name="edst2_sb")
            ostage = sb.tile([128, NH, NSUB, O], fp16, name="ostage")
            negc = sb.tile([128, 1], f32, name="negc")
            nc.vector.memset(negc[:, :], -CSH)
            zerob = sb.tile([128, 1], f32, name="zerob")
            nc.vector.memset(zerob[:, :], 0.0)
            ident = sb.tile([128, 128], f32, name="ident")
            make_identity(nc, ident[:, :])
            # warm-up exp: trigger the 1.3us ACT table load off the critical
            # path, before the first real Exp in the esrc chain
            warm = sb.tile([1, 1], f32, name="warm")
            nc.scalar.activation(
                warm[:, :], zerob[0:1, 0:1], Act.Exp, scale=1.0, bias=zerob[0:1, :]
            )

            # ---- DMA in ----
            # The kernel is HBM-DMA-wavefront bound (measured ~134 GB/s per
            # core under 8-core contention when issuing from one queue), so
            # the bulk streams are spread across THREE issuing queues
            # (sync / gpsimd / scalar) which the trace shows run in parallel,
            # and redundant bytes are trimmed: the j-columns of h belonging to
            # this core's own i-block ride only in the (early, contiguous)
            # hTi load; hTf carries the other 3072 columns.  The host applies
            # a matching j-permutation to adjT's rows (j is a contraction
            # axis, so any consistent permutation is legal).
            # all control tensors on the scalar queue too -- the sync queue is
            # starved once the other queues stream (measured: a 64 KiB wall
            # load on sync gated hp_block(0), and with it the whole DVE start)
            nc.scalar.dma_start(wta_sb, wta_d[:, :])
            nc.scalar.dma_start(
                wall_sb[:, :, 0 : NH * O],
                wr_d[:, :].rearrange("(fc p) m -> p fc m", p=128),
            )
            nc.scalar.dma_start(
                hTi_sb, hTi_d[:, :].rearrange("(fc p) i -> p fc i", p=128)
            )
            # adjT is host-pre-tiled as [8 groups, 128 partitions, 4*NI]:
            # each group DMA is 1 MiB with 8 KiB-contiguous runs per
            # partition (large-descriptor regime); even groups on the sync
            # queue, odd groups on the scalar queue, hTf thirds on gpsimd.
            # measured: the SP(sync)-issued bulk queue gets starved to
            # ~48 GB/s when the ACT/Pool-issued queues are active (~170 GB/s
            # each), so ALL bulk goes on scalar+gpsimd, interleaved in
            # consumption order; sync carries only the tiny control loads.
            adjT_r = adjT_d[:, :].rearrange("(g p) x -> g p x", p=128)
            hT_r = hTf_d[:, :].rearrange("(fc p) j -> p fc j", p=128)
            nc.scalar.dma_start(
                adjT_sb[:, 0:4, :].rearrange("p c i -> p (c i)"), adjT_r[0]
            )
            gp_seq = [("adjT", 1), ("hTf", 0), ("adjT", 3), ("hTf", 1),
                      ("adjT", 5), ("hTf", 2), ("adjT", 7)]
            for kind, idx in gp_seq:
                if kind == "hTf":
                    nc.gpsimd.dma_start(
                        hT_sb[:, :, 1024 * idx : 1024 * idx + 1024],
                        hT_r[:, :, 1024 * idx : 1024 * idx + 1024],
                    )
                else:
                    nc.gpsimd.dma_start(
                        adjT_sb[:, 4 * idx : 4 * idx + 4, :].rearrange(
                            "p c i -> p (c i)"
                        ),
                        adjT_r[idx],
                    )

            nc.vector.memset(ones_row[:, :], 1.0)

            # ---- v vectors: v[f] = sum_o wT[o,f] * a[o]  (cols: src, dst)
            # all 4 (h, fc) pairs go into one psum tile to cut scratch churn
            v_ps = pw.tile([128, 16], f32, name="v_ps", tag="scratch")
            for h in range(NH):
                for fc in range(2):
                    col = 4 * (2 * fc + h)
                    nc.tensor.matmul(
                        v_ps[:, col : col + 2],
                        lhsT=wTr_sb[:, h, ts(fc, 128)],
                        rhs=avec_sb[:, 2 * h : 2 * h + 2],
                        start=True,
                        stop=True,
                    )
            # v_ps cols are fc-major so one strided copy moves all 4 vsrc
            # values and one more the 4 vdst values into wall
            v_r = v_ps[:, :].rearrange("p (fh four) -> p fh four", four=4)
            nc.scalar.copy(
                vsrc_sb[:, :, :], v_r[:, :, 0:1].rearrange("p fh one -> p (fh one)").rearrange("p (fc h) -> p fc h", fc=2)
            )
            nc.scalar.copy(
                wall_sb[:, :, NH * O : NH * O + NH],
                v_r[:, :, 1:2].rearrange("p fh one -> p (fh one)").rearrange("p (fc h) -> p fc h", fc=2),
            )

            # ---- src row for this core's i-block, per head, then exp
            for h in range(NH):
                for seg in range(NSEG):
                    sr_ps = pw.tile([128, 512], f32, name=f"sr_ps_{h}_{seg}", tag="scratch")
                    for fc in range(2):
                        nc.tensor.matmul(
                            sr_ps[0:1, :],
                            lhsT=vsrc_sb[:, fc, h : h + 1],
                            rhs=hTi_sb[:, fc, ts(seg, 512)],
                            start=(fc == 0),
                            stop=(fc == 1),
                        )
                    nc.scalar.activation(
                        esrc3_row[:, h, ts(seg, 512)],
                        sr_ps[0:1, :],
                        Act.Exp,
                        scale=0.8,
                        bias=zerob[0:1, :],
                    )

            # deferred bulk issues: these sat ahead of the v/sr ACT ops and
            # cost ~0.7us of ACT queue each; their data is not consumed until
            # j-chunks 8/16/24, so issue them after the prologue's ACT work
            for g in (2, 4, 6):
                nc.scalar.dma_start(
                    adjT_sb[:, 4 * g : 4 * g + 4, :].rearrange("p c i -> p (c i)"),
                    adjT_r[g],
                )

            # ---- replicate e^{0.8 src} across partitions (K=1 ones matmul)
            for h in range(NH):
                for seg in range(NSEG):
                    rep_ps = pw.tile([128, 512], f32, name=f"rep_ps_{h}_{seg}", tag="scratch")
                    nc.tensor.matmul(
                        rep_ps[:, :],
                        lhsT=ones_row[:, :],
                        rhs=esrc3_row[:, h, ts(seg, 512)],
                        start=True,
                        stop=True,
                    )
                    # copy on the (still idle) DVE to keep ACT free for the
                    # hp_block exp/G chain
                    nc.vector.tensor_copy(esrc3_rep[:, h, ts(seg, 512)], rep_ps[:, :])

            # ---- bmm accumulators: psum [65, 512] per (head, i-segment)
            acc = [
                pacc.tile([O + 1, 512], f32, name=f"acc{g}", tag=f"acc{g}")
                for g in range(NH * NSEG)
            ]

            # ---- main loop over j-chunk PAIRS, with the PE/ACT producer
            # chain (h_prime + dst -> exp scalars -> G) running PIPE pairs
            # ahead of the DVE consumers.  Pairing lets one psum tile hold
            # two chunks (even at cols 0:130, odd at 256:386) so the tiny
            # per-partition exps/copies batch across both chunks+heads.
            PIPE = 2
            NPAIR = NCJ // 2
            HPW = NH * O + NH  # 130

            def hp_block(t):
                hp_ps = pw.tile([128, 512], f32, name=f"hp_ps_{t}", tag="scratch")
                for dc in range(2):
                    c = 2 * t + dc
                    # after the host j-permutation, chunks [0, NCI) of the
                    # j-axis are this core's own i-block columns (in hTi)
                    hsrc = hTi_sb if c < NCI else hT_sb
                    cc = c if c < NCI else c - NCI
                    for fc in range(2):
                        nc.tensor.matmul(
                            hp_ps[:, 256 * dc : 256 * dc + HPW],
                            lhsT=hsrc[:, fc, ts(cc, 128)],
                            rhs=wall_sb[:, fc, :],
                            start=(fc == 0),
                            stop=(fc == 1),
                        )
                # per-partition softmax scalars straight out of psum, both
                # chunks + both heads per instruction:
                # edst3 = e^{0.8 dst};  edst2 = f = e^{0.2 dst - C}
                dsts = hp_ps[:, :].rearrange("p (d x) -> p d x", d=2)[
                    :, :, NH * O : NH * O + NH
                ]
                nc.scalar.activation(
                    edst3_sb[:, 2 * t : 2 * t + 2, :],
                    dsts,
                    Act.Exp,
                    scale=0.8,
                    bias=zerob[:, :],
                )
                nc.scalar.activation(
                    edst2_sb[:, 2 * t : 2 * t + 2, :],
                    dsts,
                    Act.Exp,
                    scale=0.2,
                    bias=negc[:, :],
                )
                # G col 64 = f for both chunks+heads in one strided copy;
                # cols 0:64 = h_prime * f with f as the (fp32) scale operand
                nc.scalar.copy(
                    g2_sb[:, 2 * t : 2 * t + 2, :, O : O + 1].rearrange(
                        "p c h one -> p c (h one)"
                    ),
                    edst2_sb[:, 2 * t : 2 * t + 2, :],
                )
                for dc in range(2):
                    for h in range(NH):
                        nc.scalar.activation(
                            g2_sb[:, 2 * t + dc, h, 0:O],
                            hp_ps[:, 256 * dc + h * O : 256 * dc + (h + 1) * O],
                            Act.Copy,
                            scale=edst2_sb[:, 2 * t + dc, h : h + 1],
                        )

            for t in range(PIPE):
                hp_block(t)

            for c in range(NCJ):
                if c % 2 == 0 and c // 2 + PIPE < NPAIR:
                    hp_block(c // 2 + PIPE)
                for h in range(NH):
                    r_t = sbr.tile([128, NI], bf16, name=f"r_{h}_{c}", tag=f"R{h}", bufs=6)
                    nc.vector.tensor_scalar(
                        out=r_t[:, :],
                        in0=esrc3_rep[:, h, :],
                        scalar1=edst3_sb[:, c, h : h + 1],
                        scalar2=1.0,
                        op0=Alu.mult,
                        op1=Alu.max,
                    )
                    p_t = sbr.tile([128, NI], bf16, name=f"p_{h}_{c}", tag=f"P{h}", bufs=6)
                    nc.vector.tensor_tensor(
                        out=p_t[:, :],
                        in0=r_t[:, :],
                        in1=adjT_sb[:, c, :],
                        op=Alu.mult,
                    )
                    for seg in range(NSEG):
                        nc.tensor.matmul(
                            acc[h * NSEG + seg][:, :],
                            lhsT=g2_sb[:, c, h, 0 : O + 1],
                            rhs=p_t[:, ts(seg, 512)],
                            start=(c == 0),
                            stop=(c == NCJ - 1),
                        )

            # ---- epilogue: transpose [65,512] -> [128,65], divide, stage
            # per head, then one big DMA per head (bias is added on host)
            for h in range(NH):
                for seg in range(NSEG):
                    a_ps = acc[h * NSEG + seg]
                    tr_in = sbo.tile([O + 1, 512], f32, name=f"tr_{h}_{seg}", tag="trin")
                    nc.scalar.copy(tr_in[:, :], a_ps[:, :])
                    for q in range(4):
                        isub = seg * 4 + q
                        tr_ps = pw.tile([128, 512], f32, name=f"trp_{h}_{isub}", tag="scratch")
                        nc.tensor.transpose(
                            tr_ps[:, 0 : O + 1],
                            tr_in[:, ts(q, 128)],
                            ident[0 : O + 1, 0 : O + 1],
                        )
                        rec = sbr.tile([128, 1], f32, name=f"rec_{h}_{isub}", tag="rec")
                        nc.vector.reciprocal(rec[:, :], tr_ps[:, O : O + 1])
                        nc.scalar.activation(
                            ostage[:, h, isub, :],
                            tr_ps[:, 0:O],
                            Act.Copy,
                            scale=rec[:, :],
                        )
                nc.sync.dma_start(
                    out_d[h, :, :], ostage[:, h, :, :].rearrange("p s o -> p (s o)")
                )

    nc.finalize()
    return nc


def _prep_inputs(h, adj, w, a_src, a_dst, bias):
    """Host-side sharding / layout prep (no reference math)."""
    h = np.asarray(h, dtype=np.float32)
    adj = np.asarray(adj)
    w = np.asarray(w, dtype=np.float32)
    a_src = np.asarray(a_src, dtype=np.float32)
    a_dst = np.asarray(a_dst, dtype=np.float32)

    hT = np.ascontiguousarray(h.T)                       # [F, N]
    adjT = np.ascontiguousarray(adj.T).astype(ml_dtypes.bfloat16)  # [N, N] 0/1

    in_maps = []
    for c in range(NCORES):
        hb, ib = c % 2, c // 2
        heads = [2 * hb, 2 * hb + 1]
        i0 = NI * ib
        w2 = w[heads]                                    # [2, F, O]
        wr = np.ascontiguousarray(w2.transpose(1, 0, 2).reshape(F, NH * O))
        wTr = np.concatenate([w2[0].T, w2[1].T], axis=1)  # [O, 2F]
        avec = np.stack(
            [a_src[heads[0], :, 0], a_dst[heads[0], :, 0],
             a_src[heads[1], :, 0], a_dst[heads[1], :, 0]],
            axis=1,
        )                                                # [O, 4]
        wta = np.ascontiguousarray(np.concatenate([wTr, avec], axis=1))
        # j-permutation: this core's own i-block columns first (they ride in
        # the hTi load), remaining j's after.  adjT rows follow the same
        # permutation so the on-device chunk indexing stays consistent.
        perm = np.r_[i0 : i0 + NI, 0:i0, i0 + NI : N]
        in_maps.append(
            {
                "hTf": np.ascontiguousarray(hT[:, perm[NI:]]).astype(
                    ml_dtypes.bfloat16
                ),
                "hTi": np.ascontiguousarray(hT[:, i0 : i0 + NI]).astype(
                    ml_dtypes.bfloat16
                ),
                "adjT": np.ascontiguousarray(
                    adjT[perm, i0 : i0 + NI]
                    .reshape(8, 4, 128, NI)
                    .transpose(0, 2, 1, 3)
                    .reshape(8 * 128, 4 * NI)
                ),
                "wr": wr.astype(ml_dtypes.bfloat16),
                "wta": wta.astype(ml_dtypes.bfloat16),
            }
        )
    return in_maps


def kernel(h, adj, w, a_src, a_dst, bias):
    from concourse.bass_utils import run_bass_kernel_spmd

    if "nc" not in _CACHE:
        _CACHE["nc"] = _build()
    nc = _CACHE["nc"]

    in_maps = _prep_inputs(h, adj, w, a_src, a_dst, bias)
    res = run_bass_kernel_spmd(nc, in_maps, list(range(NCORES))).results

    out = np.empty((N, NHEAD, O), dtype=np.float32)
    for c in range(NCORES):
        hb, ib = c % 2, c // 2
        arr = np.asarray(res[c]["out"], dtype=np.float32)  # [NH, 128, NSUB*O] fp16
        for hh in range(NH):
            blk = (
                arr[hh]
                .reshape(128, NSUB, O)
                .transpose(1, 0, 2)
                .reshape(NI, O)
            )
            out[NI * ib : NI * (ib + 1), 2 * hb + hh, :] = blk
    out += np.asarray(bias, dtype=np.float32).reshape(1, 1, O)
    return out
